# revision 21
# baseline (speedup 1.0000x reference)
"""HGNNPConv (hypergraph conv, mean aggregation) on 8 Trainium2 NeuronCores.

out = leaky_relu(mean_e2v(mean_v2e(X @ W + b)))  with mean clamped to cnt>=1.

Strategy (memory-regime), v2:
  - Linearity: aggregate X first, transform at hyperedge level:
      e_feat = (mean_{v in e} X[v]) @ W + b
  - All gather payloads and matmul operands in bf16 (PSUM accumulates f32;
    rel-err budget ~0.5% << 2e-2 gate). Halves DMA bytes, and bf16 matmuls
    run ~4x faster than fp32 on the PE.
  - Gathers round-robin over 4 SWDGE queues: queue q runs on Q7 core pair
    (2q, 2q+1), so descriptor generation (the bottleneck, ~10ns/idx) runs
    4-way parallel across the 8 GPSIMD cores.
  - Phase 1 sharded by edge range: each core owns 1250 edges and all their
    incidence pairs (host-side index sort). Member X rows are fetched with
    dma_gather (int16 idx; X split into lo/hi halves at row 32768 to fit
    int16), reduced per 128-edge block via one-hot matmuls accumulating
    [feat x seg] in PSUM, scaled by 1/deg_e, then multiplied by W (features
    already on partitions, so no transposes) and biased -> e_feat rows.
  - AllGather e_feat shards (bf16) -> every core holds full [10000, 256].
  - Phase 2 sharded by vertex range: gather e_feat rows by e_idx, one-hot
    reduce [seg x feat] per 128-vertex block, scale by 1/deg_v, leaky-relu,
    write the core's [6250, 256] f32 output shard.

Host-side work is index-only (sort/bincount/packing) plus dtype casts.
"""
import sys

for _p in ("/opt/trn_rl_repo", "/opt/pypackages"):
    if _p not in sys.path:
        sys.path.insert(0, _p)

import numpy as np
import ml_dtypes

import concourse.bass as bass
import concourse.tile as tile
from concourse import bacc, mybir
from concourse.bass_utils import run_bass_kernel_spmd

NCORES = 8
NV, NE, D = 50000, 10000, 256
P = 128
E_SH = NE // NCORES      # 1250 edges per core
V_SH = NV // NCORES      # 6250 vertices per core
EB = -(-E_SH // P)       # 10 e-blocks per core
VB = -(-V_SH // P)       # 49 v-blocks per core
HALF = 32768             # int16 split point for X row indices
import os as _os
GC = int(_os.environ.get("HGNN_GC", "8"))  # max 128-pair tiles per dma_gather call
F32 = mybir.dt.float32
BF16 = mybir.dt.bfloat16
I16 = mybir.dt.int16
NPBF16 = ml_dtypes.bfloat16


def _ceil(a, b):
    return -(-a // b)


def _pack16(seq):
    """int sequence (len % 128 == 0) -> int16 SBUF image [128, len/16].

    dma_gather reads logical index k from partition k%16, free col k//16,
    with the 16-partition block replicated to all 128 partitions.
    """
    n = len(seq)
    img = np.zeros((16, n // 16), np.int16)
    img[np.arange(n) % 16, np.arange(n) // 16] = seq.astype(np.int16)
    return np.tile(img, (8, 1))


def _calls_phase1(T_lo, T_hi):
    """Per-block call layout: (tile0, ntiles, is_hi). Same for every core."""
    calls = []
    T_B = T_lo + T_hi
    for b in range(EB):
        for base, n, hi in ((b * T_B, T_lo, False), (b * T_B + T_lo, T_hi, True)):
            t = 0
            while t < n:
                c = min(GC, n - t)
                calls.append((base + t, c, hi))
                t += c
    return calls


def _calls_phase2(tiles2):
    calls = []
    t = 0
    while t < tiles2:
        c = min(GC, tiles2 - t)
        calls.append((t, c))
        t += c
    return calls



def _trim_trailing(idx, sg, calls, skip_first=0):
    """Set idx=-1 on pad lanes (sg==-1) that are trailing within their
    gather call; the gather ucode skips trailing negatives (desc-gen and
    DMA). Interior pads keep idx 0 (gather row 0, zero one-hot weight).
    skip_first: leave the first N calls untrimmed (their SBUF slots are
    uninitialized; a skipped lane there could feed NaN*0=NaN into PSUM).
    Keeps lane 0 valid so no call ends up with zero indices."""
    for ci, call in enumerate(calls):
        if ci < skip_first:
            continue
        t0, c = call[0], call[1]
        for k in range(NCORES):
            iv = idx[k, t0:t0 + c, :].reshape(-1)
            sv = sg[k, t0:t0 + c, :].reshape(-1)
            n = iv.shape[0]
            j = n
            while j > 0 and sv[j - 1] == -1.0:
                j -= 1
            if j < n:
                iv[j:] = -1
            if iv[0] == -1:
                iv[0] = 0


def _prep(v_idx, e_idx):
    """All host-side index preprocessing. Returns per-core input arrays and
    the baked program structure (uniform across cores)."""
    v_idx = np.asarray(v_idx, dtype=np.int64)
    e_idx = np.asarray(e_idx, dtype=np.int64)
    npairs = len(v_idx)

    inv_e = (1.0 / np.maximum(np.bincount(e_idx, minlength=NE), 1)).astype(np.float32)
    inv_v = (1.0 / np.maximum(np.bincount(v_idx, minlength=NV), 1)).astype(np.float32)

    # ---------------- phase 1: group pairs by (core, e-block, lo/hi) -------
    core1 = e_idx // E_SH
    eloc = e_idx - core1 * E_SH
    blk1 = core1 * EB + eloc // P
    seg1v = (eloc % P).astype(np.float32)
    is_hi = v_idx >= HALF
    key1 = blk1 * 2 + is_hi
    nruns1 = NCORES * EB * 2
    cnt1 = np.bincount(key1, minlength=nruns1)
    n_lo = cnt1[0::2].reshape(NCORES, EB)
    n_hi = cnt1[1::2].reshape(NCORES, EB)
    T_lo = int(_ceil(max(1, n_lo.max()), P))
    T_hi = int(_ceil(max(1, n_hi.max()), P))
    T_B = T_lo + T_hi
    TILES1 = EB * T_B

    order = np.argsort(key1, kind="stable")
    run_start = np.zeros(nruns1, np.int64)
    run_start[1:] = np.cumsum(cnt1)[:-1]
    pos = np.arange(npairs) - run_start[key1[order]]
    t_in_run = pos // P
    lane = pos % P
    ks = key1[order]
    hi_s = (ks % 2).astype(bool)
    blk_s = ks // 2
    core_s = blk_s // EB
    bl_s = blk_s % EB
    tile_s = bl_s * T_B + np.where(hi_s, T_lo, 0) + t_in_run
    gval = np.where(hi_s, v_idx[order] - HALF, v_idx[order])

    idx1 = np.zeros((NCORES, TILES1, P), np.int64)
    sg1 = np.full((NCORES, TILES1, P), -1.0, np.float32)
    idx1[core_s, tile_s, lane] = gval
    sg1[core_s, tile_s, lane] = seg1v[order]

    # ---------------- phase 2: group pairs by (core, v-block) --------------
    # Degree-balanced assignment: snake round-robin vertices (sorted by
    # degree desc) over all NCORES*VB blocks, so every block carries ~equal
    # pair load and T_v shrinks. vout rows become (block, lane); the host
    # inverse-permutes after gathering core outputs.
    NBLK = NCORES * VB
    deg = np.bincount(v_idx, minlength=NV)
    vorder = np.argsort(-deg, kind="stable")
    pos = np.arange(NV)
    cyc = pos % (2 * NBLK)
    snake = np.where(cyc < NBLK, cyc, 2 * NBLK - 1 - cyc)
    blk_of = np.empty(NV, np.int64)
    blk_of[vorder] = snake
    lane_rank = np.empty(NV, np.int64)
    bo = np.argsort(blk_of[vorder], kind="stable")
    tmp = vorder[bo]
    bstart = np.searchsorted(np.sort(blk_of), np.arange(NBLK))
    lane_rank[tmp] = np.arange(NV) - bstart[blk_of[tmp]]
    assert lane_rank.max() < P

    blk2 = blk_of[v_idx]
    seg2v = lane_rank[v_idx].astype(np.float32)
    core2 = blk2 // VB
    nruns2 = NCORES * VB
    cnt2 = np.bincount(blk2, minlength=nruns2)
    T_v = int(_ceil(max(1, cnt2.max()), P))
    TILES2 = VB * T_v

    order2 = np.argsort(blk2, kind="stable")
    run_start2 = np.zeros(nruns2, np.int64)
    run_start2[1:] = np.cumsum(cnt2)[:-1]
    pos2 = np.arange(npairs) - run_start2[blk2[order2]]
    t_in_run2 = pos2 // P
    lane2 = pos2 % P
    blk2_s = blk2[order2]
    core2_s = blk2_s // VB
    bl2_s = blk2_s % VB
    tile2_s = bl2_s * T_v + t_in_run2

    ec = e_idx // E_SH
    el = e_idx - ec * E_SH
    SPLIT = 1024
    e_re = np.where(el < SPLIT, ec * SPLIT + el,
                    NCORES * SPLIT + ec * (E_SH - SPLIT) + (el - SPLIT))
    idx2 = np.zeros((NCORES, TILES2, P), np.int64)
    sg2 = np.full((NCORES, TILES2, P), -1.0, np.float32)
    idx2[core2_s, tile2_s, lane2] = e_re[order2]
    sg2[core2_s, tile2_s, lane2] = seg2v[order2]

    core2_all = blk_of // VB
    row_of = core2_all * (VB * P) + (blk_of % VB) * P + lane_rank

    calls1 = _calls_phase1(T_lo, T_hi)
    calls2 = _calls_phase2(TILES2)

    # ---------------- pack per-core images ---------------------------------
    per_core = []
    for k in range(NCORES):
        g1 = np.hstack([_pack16(idx1[k, t0:t0 + c].reshape(-1))
                        for (t0, c, _hi) in calls1])
        g2 = np.hstack([_pack16(idx2[k, t0:t0 + c].reshape(-1))
                        for (t0, c) in calls2])
        s1 = np.ascontiguousarray(sg1[k].T).astype(NPBF16)     # [128, TILES1]
        s2 = np.ascontiguousarray(sg2[k].T).astype(NPBF16)     # [128, TILES2]
        ie = np.zeros(EB * P, np.float32)
        ie[:E_SH] = inv_e[k * E_SH:(k + 1) * E_SH]
        ie_img = np.tile(ie, (P, 1))                         # [128, EB*128]
        iv = np.ones(VB * P, np.float32)
        kmask = core2_all == k
        iv[(blk_of[kmask] % VB) * P + lane_rank[kmask]] = inv_v[kmask]
        iv_img = np.ascontiguousarray(iv.reshape(VB, P).T)   # [128, VB]
        per_core.append(dict(g1idx=g1, seg1=s1, g2idx=g2, seg2=s2,
                             inve=ie_img, invv=iv_img))

    struct = dict(T_lo=T_lo, T_hi=T_hi, T_v=T_v, TILES1=TILES1, TILES2=TILES2,
                  F1=per_core[0]["g1idx"].shape[1], F2=per_core[0]["g2idx"].shape[1],
                  calls1=calls1, calls2=calls2, row_of=row_of)
    return per_core, struct


def _build(st):
    """Build the SPMD bass program (identical across cores)."""
    T_B = st["T_lo"] + st["T_hi"]
    T_v = st["T_v"]
    nc = bacc.Bacc("TRN2", target_bir_lowering=False, debug=False,
                   num_devices=NCORES, num_swdge_queues=4)

    X = nc.dram_tensor("X", [NV, D], BF16, kind="ExternalInput")
    Wsb = nc.dram_tensor("Wsb", [P, 2, D], BF16, kind="ExternalInput")
    bb = nc.dram_tensor("bb", [P, D], F32, kind="ExternalInput")
    iota = nc.dram_tensor("iota", [P, P], BF16, kind="ExternalInput")
    g1idx = nc.dram_tensor("g1idx", [P, st["F1"]], I16, kind="ExternalInput")
    seg1 = nc.dram_tensor("seg1", [P, st["TILES1"]], BF16, kind="ExternalInput")
    inve = nc.dram_tensor("inve", [P, EB * P], F32, kind="ExternalInput")
    g2idx = nc.dram_tensor("g2idx", [P, st["F2"]], I16, kind="ExternalInput")
    seg2 = nc.dram_tensor("seg2", [P, st["TILES2"]], BF16, kind="ExternalInput")
    invv = nc.dram_tensor("invv", [P, VB], F32, kind="ExternalInput")
    vout = nc.dram_tensor("vout", [VB * P, D], F32, kind="ExternalOutput")


    with tile.TileContext(nc) as tc:
        with (
            tc.tile_pool(name="consts", bufs=1) as consts,
            tc.tile_pool(name="gat", bufs=8) as gat,
            tc.tile_pool(name="ohp", bufs=8) as ohp,
            tc.tile_pool(name="psp", bufs=2, space="PSUM") as psp,
            tc.tile_pool(name="psp2", bufs=4, space="PSUM") as psp2,
            tc.tile_pool(name="post", bufs=8) as post,
            tc.tile_pool(name="dram", bufs=1, space="DRAM") as dram,
        ):
            # ---- load constants / index images ----
            def load(t, shape, dt):
                s = consts.tile(shape, dt, tag=t.name)
                nc.sync.dma_start(s[:], t[:])
                return s

            W_s = load(Wsb, [P, 2, D], BF16)
            bb_s = load(bb, [P, D], F32)
            io_s = load(iota, [P, P], BF16)
            g1_s = load(g1idx, [P, st["F1"]], I16)
            s1_s = load(seg1, [P, st["TILES1"]], BF16)
            ie_s = load(inve, [P, EB * P], F32)
            g2_s = load(g2idx, [P, st["F2"]], I16)
            s2_s = load(seg2, [P, st["TILES2"]], BF16)
            iv_s = load(invv, [P, VB], F32)

            ef_local = dram.tile([E_SH, D], BF16)
            ef_all = dram.tile([NE, D], BF16)

            # ---------------- phase 1 ----------------
            X_lo = X[:]
            X_hi = X[HALF:, :]
            psum_by_block = {}
            col1 = 0

            def finish_block1(b, accA, accB):
                rows = min(P, E_SH - b * P)
                mT0 = post.tile([P, P], BF16, tag="mT")
                mT1 = post.tile([P, P], BF16, tag="mT")
                nc.vector.tensor_tensor(
                    out=mT0[:], in0=accA[:],
                    in1=ie_s[:, b * P:(b + 1) * P],
                    op=mybir.AluOpType.mult)
                nc.vector.tensor_tensor(
                    out=mT1[:], in0=accB[:],
                    in1=ie_s[:, b * P:(b + 1) * P],
                    op=mybir.AluOpType.mult)
                ef_ps = psp2.tile([P, 2 * P], F32, space="PSUM", tag="acc2")
                nc.tensor.matmul(ef_ps[:, 0:D], lhsT=mT0[:], rhs=W_s[:, 0, :],
                                 start=True, stop=False)
                nc.tensor.matmul(ef_ps[:, 0:D], lhsT=mT1[:], rhs=W_s[:, 1, :],
                                 start=False, stop=True)
                ef_sb = post.tile([P, D], BF16, tag="efsb")
                nc.vector.tensor_tensor(out=ef_sb[:], in0=ef_ps[:, 0:D], in1=bb_s[:],
                                        op=mybir.AluOpType.add)
                nc.sync.dma_start(ef_local[b * P:b * P + rows, :],
                                  ef_sb[0:rows, :])

            qn = 0
            for (t0, C, hi) in st["calls1"]:
                g = gat.tile([P, C, D], BF16, tag="g")
                nc.gpsimd.dma_gather(
                    out_ap=g[:],
                    in_ap=X_hi if hi else X_lo,
                    idxs_ap=g1_s[:, col1:col1 + C * 8],
                    num_idxs=C * P,
                    num_idxs_reg=C * P,
                    elem_size=D,
                    queue_num=qn % 4,
                )
                qn += 1
                col1 += C * 8
                oh = ohp.tile([P, C, P], BF16, tag="oh")
                nc.vector.tensor_tensor(
                    out=oh[:],
                    in0=s1_s[:, t0:t0 + C][:, :, None].to_broadcast([P, C, P]),
                    in1=io_s[:][:, None, :].to_broadcast([P, C, P]),
                    op=mybir.AluOpType.is_equal)
                for c in range(C):
                    t = t0 + c
                    b = t // T_B
                    first = (t % T_B == 0)
                    last = (t % T_B == T_B - 1)
                    if first:
                        psum_by_block[b] = (
                            psp.tile([P, P], F32, name=f"acc1a_{b}",
                                     space="PSUM", tag="accA"),
                            psp.tile([P, P], F32, name=f"acc1b_{b}",
                                     space="PSUM", tag="accB"),
                        )
                    accA, accB = psum_by_block[b]
                    nc.tensor.matmul(accA[:], lhsT=g[:, c, 0:P],
                                     rhs=oh[:, c, :], start=first, stop=last)
                    nc.tensor.matmul(accB[:], lhsT=g[:, c, P:2 * P],
                                     rhs=oh[:, c, :], start=first, stop=last)
                    if last:
                        finish_block1(b, accA, accB)
                        del psum_by_block[b]

            # ---------------- allgather e_feat (split) ----------------
            nc.gpsimd.collective_compute(
                "AllGather",
                mybir.AluOpType.bypass,
                replica_groups=[list(range(NCORES))],
                ins=[ef_local[0:1024, :].opt()],
                outs=[ef_all[0:1024 * NCORES, :].opt()],
            )
            nc.gpsimd.collective_compute(
                "AllGather",
                mybir.AluOpType.bypass,
                replica_groups=[list(range(NCORES))],
                ins=[ef_local[1024:E_SH, :].opt()],
                outs=[ef_all[1024 * NCORES:NE, :].opt()],
            )

            # ---------------- phase 2 ----------------
            col2 = 0
            psum_by_vb = {}

            def finish_block2(vb, acc):
                rows = P
                mean = post.tile([P, D], F32, tag="mean")
                nc.vector.tensor_scalar(
                    out=mean[:], in0=acc[:], scalar1=iv_s[:, vb:vb + 1],
                    scalar2=None, op0=mybir.AluOpType.mult)
                sc = post.tile([P, D], F32, tag="sc")
                nc.scalar.mul(sc[:], mean[:], 0.01)
                ot = post.tile([P, D], F32, tag="ot")
                nc.vector.tensor_tensor(out=ot[:], in0=mean[:], in1=sc[:],
                                        op=mybir.AluOpType.max)
                nc.sync.dma_start(vout[vb * P:vb * P + rows, :], ot[0:rows, :])

            for (t0, C) in st["calls2"]:
                g = gat.tile([P, C, D], BF16, tag="g")
                nc.gpsimd.dma_gather(
                    out_ap=g[:],
                    in_ap=ef_all[:],
                    idxs_ap=g2_s[:, col2:col2 + C * 8],
                    num_idxs=C * P,
                    num_idxs_reg=C * P,
                    elem_size=D,
                    queue_num=qn % 4,
                )
                qn += 1
                col2 += C * 8
                oh = ohp.tile([P, C, P], BF16, tag="oh")
                nc.vector.tensor_tensor(
                    out=oh[:],
                    in0=s2_s[:, t0:t0 + C][:, :, None].to_broadcast([P, C, P]),
                    in1=io_s[:][:, None, :].to_broadcast([P, C, P]),
                    op=mybir.AluOpType.is_equal)
                for c in range(C):
                    t = t0 + c
                    vb = t // T_v
                    first = (t % T_v == 0)
                    last = (t % T_v == T_v - 1)
                    if first:
                        psum_by_vb[vb] = psp2.tile([P, 2 * P], F32, name=f"acc2_{vb}",
                                                   space="PSUM", tag="acc2")
                    acc = psum_by_vb[vb]
                    nc.tensor.matmul(acc[:, 0:D], lhsT=oh[:, c, :],
                                     rhs=g[:, c, :], start=first, stop=last)
                    if last:
                        finish_block2(vb, acc)
                        del psum_by_vb[vb]

    nc.compile()
    return nc


def _run(inputs, trace=False, tmpdir=None):
    X = np.asarray(inputs["X"], dtype=np.float32)
    W = np.asarray(inputs["W"], dtype=np.float32)
    b = np.asarray(inputs["b"], dtype=np.float32)
    v_idx = np.asarray(inputs["v_idx"])
    e_idx = np.asarray(inputs["e_idx"])
    assert X.shape == (NV, D) and W.shape == (D, D)

    per_core, st = _prep(v_idx, e_idx)
    nc = _build(st)

    Xbf = np.ascontiguousarray(X.astype(NPBF16))
    Wsb = np.ascontiguousarray(W.reshape(2, P, D).transpose(1, 0, 2).astype(NPBF16))
    bbr = np.tile(b[None, :], (P, 1)).astype(np.float32)
    iota = np.tile(np.arange(P, dtype=np.float32), (P, 1)).astype(NPBF16)

    in_maps = []
    for k in range(NCORES):
        pc = per_core[k]
        in_maps.append({
            "X": Xbf,
            "Wsb": Wsb,
            "bb": bbr,
            "iota": iota,
            "g1idx": np.ascontiguousarray(pc["g1idx"]),
            "seg1": np.ascontiguousarray(pc["seg1"]),
            "inve": np.ascontiguousarray(pc["inve"]),
            "g2idx": np.ascontiguousarray(pc["g2idx"]),
            "seg2": np.ascontiguousarray(pc["seg2"]),
            "invv": np.ascontiguousarray(pc["invv"]),
        })

    kw = {}
    if trace:
        kw = dict(trace=True, tmpdir=tmpdir)
    res = run_bass_kernel_spmd(nc, in_maps, core_ids=list(range(NCORES)), **kw)
    cat = np.concatenate([res.results[k]["vout"] for k in range(NCORES)], axis=0)
    out = np.ascontiguousarray(cat[st["row_of"]])
    return out, res


def kernel(**inputs) -> np.ndarray:
    out, _ = _run(inputs)
    return out


# revision 23
# speedup vs baseline: 1.0439x; 1.0439x over previous
"""HGNNPConv (hypergraph conv, mean aggregation) on 8 Trainium2 NeuronCores.

out = leaky_relu(mean_e2v(mean_v2e(X @ W + b)))  with mean clamped to cnt>=1.

Strategy (memory-regime), v2:
  - Linearity: aggregate X first, transform at hyperedge level:
      e_feat = (mean_{v in e} X[v]) @ W + b
  - All gather payloads and matmul operands in bf16 (PSUM accumulates f32;
    rel-err budget ~0.5% << 2e-2 gate). Halves DMA bytes, and bf16 matmuls
    run ~4x faster than fp32 on the PE.
  - Gathers round-robin over 4 SWDGE queues: queue q runs on Q7 core pair
    (2q, 2q+1), so descriptor generation (the bottleneck, ~10ns/idx) runs
    4-way parallel across the 8 GPSIMD cores.
  - Phase 1 sharded by edge range: each core owns 1250 edges and all their
    incidence pairs (host-side index sort). Member X rows are fetched with
    dma_gather (int16 idx; X split into lo/hi halves at row 32768 to fit
    int16), reduced per 128-edge block via one-hot matmuls accumulating
    [feat x seg] in PSUM, scaled by 1/deg_e, then multiplied by W (features
    already on partitions, so no transposes) and biased -> e_feat rows.
  - AllGather e_feat shards (bf16) -> every core holds full [10000, 256].
  - Phase 2 sharded by vertex range: gather e_feat rows by e_idx, one-hot
    reduce [seg x feat] per 128-vertex block, scale by 1/deg_v, leaky-relu,
    write the core's [6250, 256] f32 output shard.

Host-side work is index-only (sort/bincount/packing) plus dtype casts.
"""
import sys

for _p in ("/opt/trn_rl_repo", "/opt/pypackages"):
    if _p not in sys.path:
        sys.path.insert(0, _p)

import numpy as np
import ml_dtypes

import concourse.bass as bass
import concourse.tile as tile
from concourse import bacc, mybir
from concourse.bass_utils import run_bass_kernel_spmd

NCORES = 8
NV, NE, D = 50000, 10000, 256
P = 128
E_SH = NE // NCORES      # 1250 edges per core
V_SH = NV // NCORES      # 6250 vertices per core
EB = -(-E_SH // P)       # 10 e-blocks per core
VB = -(-V_SH // P)       # 49 v-blocks per core
HALF = 32768             # int16 split point for X row indices
import os as _os
GC = int(_os.environ.get("HGNN_GC", "8"))  # max 128-pair tiles per dma_gather call
F32 = mybir.dt.float32
BF16 = mybir.dt.bfloat16
I16 = mybir.dt.int16
NPBF16 = ml_dtypes.bfloat16


def _ceil(a, b):
    return -(-a // b)


def _pack16(seq):
    """int sequence (len % 128 == 0) -> int16 SBUF image [128, len/16].

    dma_gather reads logical index k from partition k%16, free col k//16,
    with the 16-partition block replicated to all 128 partitions.
    """
    n = len(seq)
    img = np.zeros((16, n // 16), np.int16)
    img[np.arange(n) % 16, np.arange(n) // 16] = seq.astype(np.int16)
    return np.tile(img, (8, 1))


def _calls_phase1(T_lo, T_hi):
    """Per-block call layout: (tile0, ntiles, is_hi). Same for every core."""
    calls = []
    T_B = T_lo + T_hi
    for b in range(EB):
        for base, n, hi in ((b * T_B, T_lo, False), (b * T_B + T_lo, T_hi, True)):
            t = 0
            while t < n:
                c = min(GC, n - t)
                calls.append((base + t, c, hi))
                t += c
    return calls


def _calls_phase2(tiles2):
    calls = []
    t = 0
    while t < tiles2:
        c = min(GC, tiles2 - t)
        calls.append((t, c))
        t += c
    return calls



def _trim_trailing(idx, sg, calls, skip_first=0):
    """Set idx=-1 on pad lanes (sg==-1) that are trailing within their
    gather call; the gather ucode skips trailing negatives (desc-gen and
    DMA). Interior pads keep idx 0 (gather row 0, zero one-hot weight).
    skip_first: leave the first N calls untrimmed (their SBUF slots are
    uninitialized; a skipped lane there could feed NaN*0=NaN into PSUM).
    Keeps lane 0 valid so no call ends up with zero indices."""
    for ci, call in enumerate(calls):
        if ci < skip_first:
            continue
        t0, c = call[0], call[1]
        for k in range(NCORES):
            iv = idx[k, t0:t0 + c, :].reshape(-1)
            sv = sg[k, t0:t0 + c, :].reshape(-1)
            n = iv.shape[0]
            j = n
            while j > 0 and sv[j - 1] == -1.0:
                j -= 1
            if j < n:
                iv[j:] = -1
            if iv[0] == -1:
                iv[0] = 0


def _prep(v_idx, e_idx):
    """All host-side index preprocessing. Returns per-core input arrays and
    the baked program structure (uniform across cores)."""
    v_idx = np.asarray(v_idx, dtype=np.int64)
    e_idx = np.asarray(e_idx, dtype=np.int64)
    npairs = len(v_idx)

    inv_e = (1.0 / np.maximum(np.bincount(e_idx, minlength=NE), 1)).astype(np.float32)
    inv_v = (1.0 / np.maximum(np.bincount(v_idx, minlength=NV), 1)).astype(np.float32)

    # ---------------- phase 1: group pairs by (core, e-block, lo/hi) -------
    core1 = e_idx // E_SH
    eloc = e_idx - core1 * E_SH
    blk1 = core1 * EB + eloc // P
    seg1v = (eloc % P).astype(np.float32)
    is_hi = v_idx >= HALF
    key1 = blk1 * 2 + is_hi
    nruns1 = NCORES * EB * 2
    cnt1 = np.bincount(key1, minlength=nruns1)
    n_lo = cnt1[0::2].reshape(NCORES, EB)
    n_hi = cnt1[1::2].reshape(NCORES, EB)
    T_lo = int(_ceil(max(1, n_lo.max()), P))
    T_hi = int(_ceil(max(1, n_hi.max()), P))
    T_B = T_lo + T_hi
    TILES1 = EB * T_B

    order = np.lexsort((v_idx, key1))
    run_start = np.zeros(nruns1, np.int64)
    run_start[1:] = np.cumsum(cnt1)[:-1]
    pos = np.arange(npairs) - run_start[key1[order]]
    t_in_run = pos // P
    lane = pos % P
    ks = key1[order]
    hi_s = (ks % 2).astype(bool)
    blk_s = ks // 2
    core_s = blk_s // EB
    bl_s = blk_s % EB
    tile_s = bl_s * T_B + np.where(hi_s, T_lo, 0) + t_in_run
    gval = np.where(hi_s, v_idx[order] - HALF, v_idx[order])

    idx1 = np.zeros((NCORES, TILES1, P), np.int64)
    sg1 = np.full((NCORES, TILES1, P), -1.0, np.float32)
    idx1[core_s, tile_s, lane] = gval
    sg1[core_s, tile_s, lane] = seg1v[order]

    # ---------------- phase 2: group pairs by (core, v-block) --------------
    # Degree-balanced assignment: snake round-robin vertices (sorted by
    # degree desc) over all NCORES*VB blocks, so every block carries ~equal
    # pair load and T_v shrinks. vout rows become (block, lane); the host
    # inverse-permutes after gathering core outputs.
    NBLK = NCORES * VB
    deg = np.bincount(v_idx, minlength=NV)
    vorder = np.argsort(-deg, kind="stable")
    pos = np.arange(NV)
    cyc = pos % (2 * NBLK)
    snake = np.where(cyc < NBLK, cyc, 2 * NBLK - 1 - cyc)
    blk_of = np.empty(NV, np.int64)
    blk_of[vorder] = snake
    lane_rank = np.empty(NV, np.int64)
    bo = np.argsort(blk_of[vorder], kind="stable")
    tmp = vorder[bo]
    bstart = np.searchsorted(np.sort(blk_of), np.arange(NBLK))
    lane_rank[tmp] = np.arange(NV) - bstart[blk_of[tmp]]
    assert lane_rank.max() < P

    blk2 = blk_of[v_idx]
    seg2v = lane_rank[v_idx].astype(np.float32)
    core2 = blk2 // VB
    nruns2 = NCORES * VB
    cnt2 = np.bincount(blk2, minlength=nruns2)
    T_v = int(_ceil(max(1, cnt2.max()), P))
    TILES2 = VB * T_v

    order2 = np.lexsort((e_idx, blk2))
    run_start2 = np.zeros(nruns2, np.int64)
    run_start2[1:] = np.cumsum(cnt2)[:-1]
    pos2 = np.arange(npairs) - run_start2[blk2[order2]]
    t_in_run2 = pos2 // P
    lane2 = pos2 % P
    blk2_s = blk2[order2]
    core2_s = blk2_s // VB
    bl2_s = blk2_s % VB
    tile2_s = bl2_s * T_v + t_in_run2

    idx2 = np.zeros((NCORES, TILES2, P), np.int64)
    sg2 = np.full((NCORES, TILES2, P), -1.0, np.float32)
    idx2[core2_s, tile2_s, lane2] = e_idx[order2]
    sg2[core2_s, tile2_s, lane2] = seg2v[order2]

    core2_all = blk_of // VB
    row_of = core2_all * (VB * P) + (blk_of % VB) * P + lane_rank

    calls1 = _calls_phase1(T_lo, T_hi)
    calls2 = _calls_phase2(TILES2)

    # ---------------- pack per-core images ---------------------------------
    per_core = []
    for k in range(NCORES):
        g1 = np.hstack([_pack16(idx1[k, t0:t0 + c].reshape(-1))
                        for (t0, c, _hi) in calls1])
        g2 = np.hstack([_pack16(idx2[k, t0:t0 + c].reshape(-1))
                        for (t0, c) in calls2])
        s1 = np.ascontiguousarray(sg1[k].T).astype(NPBF16)     # [128, TILES1]
        s2 = np.ascontiguousarray(sg2[k].T).astype(NPBF16)     # [128, TILES2]
        ie = np.zeros(EB * P, np.float32)
        ie[:E_SH] = inv_e[k * E_SH:(k + 1) * E_SH]
        ie_img = np.tile(ie, (P, 1))                         # [128, EB*128]
        iv = np.ones(VB * P, np.float32)
        kmask = core2_all == k
        iv[(blk_of[kmask] % VB) * P + lane_rank[kmask]] = inv_v[kmask]
        iv_img = np.ascontiguousarray(iv.reshape(VB, P).T)   # [128, VB]
        per_core.append(dict(g1idx=g1, seg1=s1, g2idx=g2, seg2=s2,
                             inve=ie_img, invv=iv_img))

    struct = dict(T_lo=T_lo, T_hi=T_hi, T_v=T_v, TILES1=TILES1, TILES2=TILES2,
                  F1=per_core[0]["g1idx"].shape[1], F2=per_core[0]["g2idx"].shape[1],
                  calls1=calls1, calls2=calls2, row_of=row_of)
    return per_core, struct


def _build(st):
    """Build the SPMD bass program (identical across cores)."""
    T_B = st["T_lo"] + st["T_hi"]
    T_v = st["T_v"]
    nc = bacc.Bacc("TRN2", target_bir_lowering=False, debug=False,
                   num_devices=NCORES, num_swdge_queues=4)

    X = nc.dram_tensor("X", [NV, D], BF16, kind="ExternalInput")
    Wsb = nc.dram_tensor("Wsb", [P, 2, D], BF16, kind="ExternalInput")
    bb = nc.dram_tensor("bb", [P, D], F32, kind="ExternalInput")
    iota = nc.dram_tensor("iota", [P, P], BF16, kind="ExternalInput")
    g1idx = nc.dram_tensor("g1idx", [P, st["F1"]], I16, kind="ExternalInput")
    seg1 = nc.dram_tensor("seg1", [P, st["TILES1"]], BF16, kind="ExternalInput")
    inve = nc.dram_tensor("inve", [P, EB * P], F32, kind="ExternalInput")
    g2idx = nc.dram_tensor("g2idx", [P, st["F2"]], I16, kind="ExternalInput")
    seg2 = nc.dram_tensor("seg2", [P, st["TILES2"]], BF16, kind="ExternalInput")
    invv = nc.dram_tensor("invv", [P, VB], F32, kind="ExternalInput")
    vout = nc.dram_tensor("vout", [VB * P, D], F32, kind="ExternalOutput")


    with tile.TileContext(nc) as tc:
        with (
            tc.tile_pool(name="consts", bufs=1) as consts,
            tc.tile_pool(name="gat", bufs=8) as gat,
            tc.tile_pool(name="ohp", bufs=8) as ohp,
            tc.tile_pool(name="psp", bufs=2, space="PSUM") as psp,
            tc.tile_pool(name="psp2", bufs=4, space="PSUM") as psp2,
            tc.tile_pool(name="post", bufs=8) as post,
            tc.tile_pool(name="dram", bufs=1, space="DRAM") as dram,
        ):
            # ---- load constants / index images ----
            def load(t, shape, dt):
                s = consts.tile(shape, dt, tag=t.name)
                nc.sync.dma_start(s[:], t[:])
                return s

            g1_s = load(g1idx, [P, st["F1"]], I16)
            io_s = load(iota, [P, P], BF16)
            s1_s = load(seg1, [P, st["TILES1"]], BF16)
            W_s = load(Wsb, [P, 2, D], BF16)
            bb_s = load(bb, [P, D], F32)
            ie_s = load(inve, [P, EB * P], F32)
            g2_s = load(g2idx, [P, st["F2"]], I16)
            s2_s = load(seg2, [P, st["TILES2"]], BF16)
            iv_s = load(invv, [P, VB], F32)

            ef_local = dram.tile([E_SH, D], BF16)
            ef_all = dram.tile([NE, D], BF16)

            # ---------------- phase 1 ----------------
            X_lo = X[:]
            X_hi = X[HALF:, :]
            psum_by_block = {}
            col1 = 0

            def finish_block1(b, accA, accB):
                rows = min(P, E_SH - b * P)
                mT0 = post.tile([P, P], BF16, tag="mT")
                mT1 = post.tile([P, P], BF16, tag="mT")
                nc.vector.tensor_tensor(
                    out=mT0[:], in0=accA[:],
                    in1=ie_s[:, b * P:(b + 1) * P],
                    op=mybir.AluOpType.mult)
                nc.vector.tensor_tensor(
                    out=mT1[:], in0=accB[:],
                    in1=ie_s[:, b * P:(b + 1) * P],
                    op=mybir.AluOpType.mult)
                ef_ps = psp2.tile([P, 2 * P], F32, space="PSUM", tag="acc2")
                nc.tensor.matmul(ef_ps[:, 0:D], lhsT=mT0[:], rhs=W_s[:, 0, :],
                                 start=True, stop=False)
                nc.tensor.matmul(ef_ps[:, 0:D], lhsT=mT1[:], rhs=W_s[:, 1, :],
                                 start=False, stop=True)
                ef_sb = post.tile([P, D], BF16, tag="efsb")
                nc.vector.tensor_tensor(out=ef_sb[:], in0=ef_ps[:, 0:D], in1=bb_s[:],
                                        op=mybir.AluOpType.add)
                nc.sync.dma_start(ef_local[b * P:b * P + rows, :],
                                  ef_sb[0:rows, :])

            qn = 0
            for (t0, C, hi) in st["calls1"]:
                g = gat.tile([P, C, D], BF16, tag="g")
                nc.gpsimd.dma_gather(
                    out_ap=g[:],
                    in_ap=X_hi if hi else X_lo,
                    idxs_ap=g1_s[:, col1:col1 + C * 8],
                    num_idxs=C * P,
                    num_idxs_reg=C * P,
                    elem_size=D,
                    queue_num=qn % 4,
                )
                qn += 1
                col1 += C * 8
                oh = ohp.tile([P, C, P], BF16, tag="oh")
                nc.vector.tensor_tensor(
                    out=oh[:],
                    in0=s1_s[:, t0:t0 + C][:, :, None].to_broadcast([P, C, P]),
                    in1=io_s[:][:, None, :].to_broadcast([P, C, P]),
                    op=mybir.AluOpType.is_equal)
                for c in range(C):
                    t = t0 + c
                    b = t // T_B
                    first = (t % T_B == 0)
                    last = (t % T_B == T_B - 1)
                    if first:
                        psum_by_block[b] = (
                            psp.tile([P, P], F32, name=f"acc1a_{b}",
                                     space="PSUM", tag="accA"),
                            psp.tile([P, P], F32, name=f"acc1b_{b}",
                                     space="PSUM", tag="accB"),
                        )
                    accA, accB = psum_by_block[b]
                    nc.tensor.matmul(accA[:], lhsT=g[:, c, 0:P],
                                     rhs=oh[:, c, :], start=first, stop=last)
                    nc.tensor.matmul(accB[:], lhsT=g[:, c, P:2 * P],
                                     rhs=oh[:, c, :], start=first, stop=last)
                    if last:
                        finish_block1(b, accA, accB)
                        del psum_by_block[b]

            # ---------------- allgather e_feat ----------------
            nc.gpsimd.collective_compute(
                "AllGather",
                mybir.AluOpType.bypass,
                replica_groups=[list(range(NCORES))],
                ins=[ef_local[:].opt()],
                outs=[ef_all[:].opt()],
            )

            # ---------------- phase 2 ----------------
            col2 = 0
            psum_by_vb = {}

            def finish_block2(vb, acc):
                rows = P
                mean = post.tile([P, D], F32, tag="mean")
                nc.vector.tensor_scalar(
                    out=mean[:], in0=acc[:], scalar1=iv_s[:, vb:vb + 1],
                    scalar2=None, op0=mybir.AluOpType.mult)
                sc = post.tile([P, D], F32, tag="sc")
                nc.scalar.mul(sc[:], mean[:], 0.01)
                ot = post.tile([P, D], F32, tag="ot")
                nc.vector.tensor_tensor(out=ot[:], in0=mean[:], in1=sc[:],
                                        op=mybir.AluOpType.max)
                nc.sync.dma_start(vout[vb * P:vb * P + rows, :], ot[0:rows, :])

            for (t0, C) in st["calls2"]:
                g = gat.tile([P, C, D], BF16, tag="g")
                nc.gpsimd.dma_gather(
                    out_ap=g[:],
                    in_ap=ef_all[:],
                    idxs_ap=g2_s[:, col2:col2 + C * 8],
                    num_idxs=C * P,
                    num_idxs_reg=C * P,
                    elem_size=D,
                    queue_num=qn % 4,
                )
                qn += 1
                col2 += C * 8
                oh = ohp.tile([P, C, P], BF16, tag="oh")
                nc.vector.tensor_tensor(
                    out=oh[:],
                    in0=s2_s[:, t0:t0 + C][:, :, None].to_broadcast([P, C, P]),
                    in1=io_s[:][:, None, :].to_broadcast([P, C, P]),
                    op=mybir.AluOpType.is_equal)
                for c in range(C):
                    t = t0 + c
                    vb = t // T_v
                    first = (t % T_v == 0)
                    last = (t % T_v == T_v - 1)
                    if first:
                        psum_by_vb[vb] = psp2.tile([P, 2 * P], F32, name=f"acc2_{vb}",
                                                   space="PSUM", tag="acc2")
                    acc = psum_by_vb[vb]
                    nc.tensor.matmul(acc[:, 0:D], lhsT=oh[:, c, :],
                                     rhs=g[:, c, :], start=first, stop=last)
                    if last:
                        finish_block2(vb, acc)
                        del psum_by_vb[vb]

    nc.compile()
    return nc


def _run(inputs, trace=False, tmpdir=None):
    X = np.asarray(inputs["X"], dtype=np.float32)
    W = np.asarray(inputs["W"], dtype=np.float32)
    b = np.asarray(inputs["b"], dtype=np.float32)
    v_idx = np.asarray(inputs["v_idx"])
    e_idx = np.asarray(inputs["e_idx"])
    assert X.shape == (NV, D) and W.shape == (D, D)

    per_core, st = _prep(v_idx, e_idx)
    nc = _build(st)

    Xbf = np.ascontiguousarray(X.astype(NPBF16))
    Wsb = np.ascontiguousarray(W.reshape(2, P, D).transpose(1, 0, 2).astype(NPBF16))
    bbr = np.tile(b[None, :], (P, 1)).astype(np.float32)
    iota = np.tile(np.arange(P, dtype=np.float32), (P, 1)).astype(NPBF16)

    in_maps = []
    for k in range(NCORES):
        pc = per_core[k]
        in_maps.append({
            "X": Xbf,
            "Wsb": Wsb,
            "bb": bbr,
            "iota": iota,
            "g1idx": np.ascontiguousarray(pc["g1idx"]),
            "seg1": np.ascontiguousarray(pc["seg1"]),
            "inve": np.ascontiguousarray(pc["inve"]),
            "g2idx": np.ascontiguousarray(pc["g2idx"]),
            "seg2": np.ascontiguousarray(pc["seg2"]),
            "invv": np.ascontiguousarray(pc["invv"]),
        })

    kw = {}
    if trace:
        kw = dict(trace=True, tmpdir=tmpdir)
    res = run_bass_kernel_spmd(nc, in_maps, core_ids=list(range(NCORES)), **kw)
    cat = np.concatenate([res.results[k]["vout"] for k in range(NCORES)], axis=0)
    out = np.ascontiguousarray(cat[st["row_of"]])
    return out, res


def kernel(**inputs) -> np.ndarray:
    out, _ = _run(inputs)
    return out
